# revision 1
# baseline (speedup 1.0000x reference)
"""Trainium2 Bass kernel for nn_H_H_EdgeApplyModule (GNN edge-apply).

Reference computation:
    feat      = concat([n_f[src], s_f, n_f[dst]], 1)          # [E, 3072]
    feat_lang = concat([word2vec[src], word2vec[dst]], 1)     # [E, 600]
    e_f       = relu(feat @ W1 + b1)                          # [E, 256]
    e_f_lang  = relu(feat_lang @ Wl + bl)                     # [E, 256]

Algebraic restructure (cuts FLOPs 2.7x and gather bytes 2.4x):
    W1 = [W1a; W1b; W1c] (rows 0:1024, 1024:2048, 2048:3072)
    Wl = [Wla; Wlb]      (rows 0:300, 300:600)
    P  = n_f @ W1a + b1   Q  = n_f @ W1c
    Pl = w2v @ Wla + bl   Ql = w2v @ Wlb
    e_f      = relu(P[src] + s_f @ W1b + Q[dst])
    e_f_lang = relu(Pl[src] + Ql[dst])

Distribution (8 cores):
    - Node tables: each core computes a 1/8 shard of Tsrc=[P|Pl] and
      Tdst=[Q|Ql], then AllGather -> full tables in local DRAM.
    - Edges: sharded contiguously; each core handles E/8 edges with
      dma_gather (row gather by edge index) + PE matmuls.
"""

import sys

sys.path.insert(0, "/opt/trn_rl_repo")

import numpy as np

from concourse import bass, bacc, tile, mybir
from concourse.bass_utils import run_bass_kernel_spmd

F32 = mybir.dt.float32
F32R = mybir.dt.float32r
F16 = mybir.dt.float16
I16 = mybir.dt.int16

# ---------------------------------------------------------------- config
N_CORES = 8
N_NODES = 16384
E_TOTAL = 131072
D = 1024          # node/spatial feature dim
DW_PAD = 384      # word2vec dim padded 300 -> 384 (3 full 128-chunks)
DOUT = 256
TBL = 512         # table row: [P|Pl] or [Q|Ql]

E_CORE = E_TOTAL // N_CORES          # 16384
NODE_SHARD = N_NODES // N_CORES      # 2048
EDGE_TILE = 128
BATCH_TILES = 8                      # edge tiles per gather batch
BATCH = EDGE_TILE * BATCH_TILES      # 1024 edges per gather
KC_D = D // 128                      # 8 K-chunks for 1024-dim features
KC_W = DW_PAD // 128                 # 3 K-chunks for word2vec

# dtype of the gathered node tables in DRAM (F32 safe, F16 halves traffic)
TABLE_DT = F16
# dtype used for the s_f matmul path (F32R: full-rate fp32-ish matmul)
MM_DT = F16


def build_kernel(n_cores=N_CORES, node_shard=NODE_SHARD, e_core=E_CORE,
                 batch_tiles=BATCH_TILES, table_dt=TABLE_DT):
    n_nodes = node_shard * n_cores
    batch = EDGE_TILE * batch_tiles
    n_batches = e_core // batch
    node_tiles = node_shard // 128
    idx_cols = e_core // 16

    nc = bacc.Bacc("TRN2", target_bir_lowering=False, debug=False,
                   num_devices=n_cores)

    # ---------------- I/O ----------------
    nf_sh = nc.declare_dram_parameter("nf_shard", [node_shard, D], F32, isOutput=False)
    w2v_sh = nc.declare_dram_parameter("w2v_shard", [node_shard, DW_PAD], F32, isOutput=False)
    sf = nc.declare_dram_parameter("sf", [e_core, D], F32, isOutput=False)
    w_nf = nc.declare_dram_parameter("w_nf", [D, TBL], F16, isOutput=False)       # [W1a|W1c]
    w_l = nc.declare_dram_parameter("w_l", [DW_PAD, TBL], F16, isOutput=False)    # [Wla|Wlb]
    w1b = nc.declare_dram_parameter("w1b", [D, DOUT], F16, isOutput=False)
    bias = nc.declare_dram_parameter("bias_src", [1, TBL], F32, isOutput=False)   # [b1|bl]
    ones = nc.declare_dram_parameter("ones", [1, 128], F32, isOutput=False)
    ident = nc.declare_dram_parameter("identity", [128, 128], F32, isOutput=False)
    ident_h = nc.declare_dram_parameter("identity_h", [128, 128], table_dt, isOutput=False)
    idx_src = nc.declare_dram_parameter("idx_src", [128, idx_cols], I16, isOutput=False)
    idx_dst = nc.declare_dram_parameter("idx_dst", [128, idx_cols], I16, isOutput=False)
    out_e = nc.declare_dram_parameter("out_e", [e_core, DOUT], F32,
                                      isOutput=True)
    out_l = nc.declare_dram_parameter("out_l", [e_core, DOUT], F32,
                                      isOutput=True)

    # ---------------- internal DRAM ----------------
    tsrc_sh = nc.dram_tensor("tsrc_shard", [node_shard, TBL], table_dt)
    tdst_sh = nc.dram_tensor("tdst_shard", [node_shard, TBL], table_dt)
    tsrc = nc.dram_tensor("tsrc_full", [n_nodes, TBL], table_dt,
                          addr_space="Shared")
    tdst = nc.dram_tensor("tdst_full", [n_nodes, TBL], table_dt,
                          addr_space="Shared")

    with tile.TileContext(nc) as tc:
        with (
            tc.tile_pool(name="const", bufs=1) as cpool,
            tc.tile_pool(name="psum_b", bufs=1, space="PSUM") as pbias,
        ):
            # persistent constants in SBUF
            w_nf_sb = cpool.tile([128, KC_D, TBL], F16)
            nc.sync.dma_start(w_nf_sb[:], w_nf[:].rearrange("(c p) n -> p c n", p=128))
            w_l_sb = cpool.tile([128, KC_W, TBL], F16)
            nc.sync.dma_start(w_l_sb[:], w_l[:].rearrange("(c p) n -> p c n", p=128))
            w1b_sb = cpool.tile([128, KC_D, DOUT], F16)
            nc.sync.dma_start(w1b_sb[:], w1b[:].rearrange("(c p) n -> p c n", p=128))
            ident_sb = cpool.tile([128, 128], F32)
            nc.sync.dma_start(ident_sb[:], ident[:])
            ident_h_sb = cpool.tile([128, 128], table_dt)
            nc.sync.dma_start(ident_h_sb[:], ident_h[:])
            ones_sb = cpool.tile([1, 128], F32)
            nc.sync.dma_start(ones_sb[:], ones[:])
            bias_sb = cpool.tile([1, TBL], F32)
            nc.sync.dma_start(bias_sb[:], bias[:])
            idx_src_sb = cpool.tile([128, idx_cols], I16)
            nc.sync.dma_start(idx_src_sb[:], idx_src[:])
            idx_dst_sb = cpool.tile([128, idx_cols], I16)
            nc.sync.dma_start(idx_dst_sb[:], idx_dst[:])

            # broadcast bias to all 128 partitions: psum = ones.T @ bias
            bias_full = cpool.tile([128, TBL], F32)
            pb = pbias.tile([128, TBL], F32)
            nc.tensor.matmul(pb[:], ones_sb[:], bias_sb[:], start=True, stop=True)
            nc.vector.tensor_copy(bias_full[:], pb[:])

            # ============ phase 1: node tables (sharded) ============
            with (
                tc.tile_pool(name="p1_in", bufs=2) as p1in,
                tc.tile_pool(name="p1_t", bufs=2) as p1t,
                tc.tile_pool(name="p1_out", bufs=2) as p1out,
                tc.tile_pool(name="p1_ptr", bufs=2, space="PSUM") as p1ptr,
                tc.tile_pool(name="p1_psrc", bufs=2, space="PSUM") as p1psrc,
                tc.tile_pool(name="p1_pdst", bufs=2, space="PSUM") as p1pdst,
            ):
                for nt in range(node_tiles):
                    r0 = nt * 128
                    nf_t = p1in.tile([128, D], F32, tag="nf")
                    nc.sync.dma_start(nf_t[:], nf_sh[r0:r0 + 128, :])
                    w2v_t = p1in.tile([128, DW_PAD], F32, tag="w2v")
                    nc.sync.dma_start(w2v_t[:], w2v_sh[r0:r0 + 128, :])

                    # transpose node features: features -> partitions
                    nfT = p1t.tile([128, KC_D, 128], F16, tag="nfT")
                    for g in range(KC_D // 4):
                        ptr = p1ptr.tile([128, 4, 128], F32)
                        for j in range(4):
                            kc = g * 4 + j
                            nc.tensor.transpose(
                                ptr[:, j, :],
                                nf_t[:, kc * 128:(kc + 1) * 128], ident_sb[:])
                        nc.vector.tensor_copy(nfT[:, g * 4:(g + 1) * 4, :], ptr[:])
                    w2vT = p1t.tile([128, KC_W, 128], F16, tag="w2vT")
                    ptr = p1ptr.tile([128, 4, 128], F32)
                    for kc in range(KC_W):
                        nc.tensor.transpose(
                            ptr[:, kc, :],
                            w2v_t[:, kc * 128:(kc + 1) * 128], ident_sb[:])
                    nc.vector.tensor_copy(w2vT[:, 0:KC_W, :], ptr[:, 0:KC_W, :])

                    # Tsrc = [P | Pl] + [b1|bl],  Tdst = [Q | Ql]
                    ps = p1psrc.tile([128, TBL], F32)
                    pd = p1pdst.tile([128, TBL], F32)
                    for kc in range(KC_D):
                        nc.tensor.matmul(
                            ps[:, 0:DOUT],
                            nfT[:, kc, :],
                            w_nf_sb[:, kc, 0:DOUT],
                            start=(kc == 0), stop=(kc == KC_D - 1))
                    for kc in range(KC_W):
                        nc.tensor.matmul(
                            ps[:, DOUT:TBL],
                            w2vT[:, kc, :],
                            w_l_sb[:, kc, 0:DOUT],
                            start=(kc == 0), stop=(kc == KC_W - 1))
                    for kc in range(KC_D):
                        nc.tensor.matmul(
                            pd[:, 0:DOUT],
                            nfT[:, kc, :],
                            w_nf_sb[:, kc, DOUT:TBL],
                            start=(kc == 0), stop=(kc == KC_D - 1))
                    for kc in range(KC_W):
                        nc.tensor.matmul(
                            pd[:, DOUT:TBL],
                            w2vT[:, kc, :],
                            w_l_sb[:, kc, DOUT:TBL],
                            start=(kc == 0), stop=(kc == KC_W - 1))

                    src_o = p1out.tile([128, TBL], table_dt, tag="src_o")
                    dst_o = p1out.tile([128, TBL], table_dt, tag="dst_o")
                    nc.vector.tensor_add(src_o[:], ps[:], bias_full[:])
                    nc.scalar.copy(dst_o[:], pd[:])
                    nc.sync.dma_start(tsrc_sh[r0:r0 + 128, :], src_o[:])
                    nc.sync.dma_start(tdst_sh[r0:r0 + 128, :], dst_o[:])

            # ============ AllGather tables across cores ============
            groups = [list(range(n_cores))]
            nc.gpsimd.collective_compute(
                "AllGather", mybir.AluOpType.bypass, replica_groups=groups,
                ins=[tsrc_sh[:]], outs=[tsrc[:]])
            nc.gpsimd.collective_compute(
                "AllGather", mybir.AluOpType.bypass, replica_groups=groups,
                ins=[tdst_sh[:]], outs=[tdst[:]])

            # ============ phase 2: edges ============
            with (
                tc.tile_pool(name="p2_sf", bufs=3) as p2sf,
                tc.tile_pool(name="p2_sfT", bufs=3) as p2sft,
                tc.tile_pool(name="p2_g", bufs=3) as p2g,
                tc.tile_pool(name="p2_out", bufs=4) as p2out,
                tc.tile_pool(name="p2_ptr", bufs=3, space="PSUM") as p2ptr,
                tc.tile_pool(name="p2_pe", bufs=2, space="PSUM") as p2pe,
                tc.tile_pool(name="p2_pl", bufs=2, space="PSUM") as p2pl,
            ):
                for b in range(n_batches):
                    c0 = b * (batch // 16)
                    g_src = p2g.tile([128, batch_tiles, TBL], table_dt, tag="gs")
                    nc.gpsimd.dma_gather(
                        g_src[:], tsrc[:], idx_src_sb[:, c0:c0 + batch // 16],
                        batch, batch, TBL)
                    g_dst = p2g.tile([128, batch_tiles, TBL], table_dt, tag="gd")
                    nc.gpsimd.dma_gather(
                        g_dst[:], tdst[:], idx_dst_sb[:, c0:c0 + batch // 16],
                        batch, batch, TBL)

                    for t in range(batch_tiles):
                        e0 = (b * batch_tiles + t) * EDGE_TILE
                        sf_t = p2sf.tile([128, D], F32, tag="sf")
                        nc.sync.dma_start(sf_t[:], sf[e0:e0 + 128, :])
                        sf16 = p2sf.tile([128, D], F16, tag="sf16")
                        nc.scalar.copy(sf16[:], sf_t[:])

                        sfT = p2sft.tile([128, KC_D, 128], F16, tag="sfT")
                        for g in range(KC_D // 4):
                            ptr = p2ptr.tile([128, 4, 128], F16)
                            for j in range(4):
                                kc = g * 4 + j
                                nc.tensor.transpose(
                                    ptr[:, j, :],
                                    sf16[:, kc * 128:(kc + 1) * 128],
                                    ident_h_sb[:])
                            nc.vector.tensor_copy(
                                sfT[:, g * 4:(g + 1) * 4, :], ptr[:])

                        pe = p2pe.tile([128, DOUT], F32)
                        for kc in range(KC_D):
                            nc.tensor.matmul(
                                pe[:],
                                sfT[:, kc, :],
                                w1b_sb[:, kc, :],
                                start=(kc == 0), stop=False)
                        nc.tensor.matmul(pe[:], ident_h_sb[:],
                                         g_src[:, t, 0:DOUT],
                                         start=False, stop=False)
                        nc.tensor.matmul(pe[:], ident_h_sb[:],
                                         g_dst[:, t, 0:DOUT],
                                         start=False, stop=True)

                        pl = p2pl.tile([128, DOUT], F32)
                        nc.tensor.matmul(pl[:], ident_h_sb[:],
                                         g_src[:, t, DOUT:TBL],
                                         start=True, stop=False)
                        nc.tensor.matmul(pl[:], ident_h_sb[:],
                                         g_dst[:, t, DOUT:TBL],
                                         start=False, stop=True)

                        oe = p2out.tile([128, DOUT], F32, tag="oe")
                        ol = p2out.tile([128, DOUT], F32, tag="ol")
                        nc.scalar.activation(
                            oe[:], pe[:], mybir.ActivationFunctionType.Relu)
                        nc.scalar.activation(
                            ol[:], pl[:], mybir.ActivationFunctionType.Relu)
                        nc.sync.dma_start(out_e[e0:e0 + 128, :], oe[:])
                        nc.sync.dma_start(out_l[e0:e0 + 128, :], ol[:])

    nc.compile()
    return nc


# ---------------------------------------------------------------- host side
def _wrap_idx(ix, batch):
    """int16 index layout for dma_gather: idx j of a batch sits at
    (partition j%16, column j//16); 16-row block replicated to 128."""
    e = ix.shape[0]
    n_b = e // batch
    cols = batch // 16
    arr = np.zeros((16, e // 16), dtype=np.int16)
    for b in range(n_b):
        blk = ix[b * batch:(b + 1) * batch].astype(np.int16).reshape(cols, 16).T
        arr[:, b * cols:(b + 1) * cols] = blk
    return np.ascontiguousarray(np.tile(arr, (8, 1)))


_NC_CACHE = {}


def make_in_maps(n_f, word2vec, s_f, W1, b1, Wl, bl, src, dst):
    n_f = np.asarray(n_f, dtype=np.float32)
    word2vec = np.asarray(word2vec, dtype=np.float32)
    s_f = np.asarray(s_f, dtype=np.float32)
    W1 = np.asarray(W1, dtype=np.float32)
    Wl = np.asarray(Wl, dtype=np.float32)
    b1 = np.asarray(b1, dtype=np.float32)
    bl = np.asarray(bl, dtype=np.float32)
    src = np.asarray(src)
    dst = np.asarray(dst)

    w2v_pad = np.zeros((N_NODES, DW_PAD), np.float32)
    w2v_pad[:, :300] = word2vec
    w_nf = np.ascontiguousarray(
        np.concatenate([W1[0:D], W1[2 * D:3 * D]], axis=1)).astype(np.float16)
    w_l = np.zeros((DW_PAD, TBL), np.float16)
    w_l[:300, 0:DOUT] = Wl[0:300]
    w_l[:300, DOUT:TBL] = Wl[300:600]
    w1b = np.ascontiguousarray(W1[D:2 * D]).astype(np.float16)
    bias_src = np.concatenate([b1, bl])[None, :].astype(np.float32)
    ones = np.ones((1, 128), np.float32)
    ident = np.eye(128, dtype=np.float32)
    ident_h = np.eye(128, dtype=mybir.dt.np(TABLE_DT))

    in_maps = []
    for k in range(N_CORES):
        es, ee = k * E_CORE, (k + 1) * E_CORE
        ns, ne = k * NODE_SHARD, (k + 1) * NODE_SHARD
        in_maps.append({
            "nf_shard": np.ascontiguousarray(n_f[ns:ne]),
            "w2v_shard": np.ascontiguousarray(w2v_pad[ns:ne]),
            "sf": np.ascontiguousarray(s_f[es:ee]),
            "w_nf": w_nf,
            "w_l": w_l,
            "w1b": w1b,
            "bias_src": bias_src,
            "ones": ones,
            "identity": ident,
            "identity_h": ident_h,
            "idx_src": _wrap_idx(src[es:ee], BATCH),
            "idx_dst": _wrap_idx(dst[es:ee], BATCH),
        })

    return in_maps


def kernel(n_f, word2vec, s_f, W1, b1, Wl, bl, src, dst):
    if "nc" not in _NC_CACHE:
        _NC_CACHE["nc"] = build_kernel()
    nc = _NC_CACHE["nc"]
    in_maps = make_in_maps(n_f, word2vec, s_f, W1, b1, Wl, bl, src, dst)
    res = run_bass_kernel_spmd(nc, in_maps, list(range(N_CORES)))
    _NC_CACHE["last_results"] = res
    e_f = np.concatenate([res.results[k]["out_e"] for k in range(N_CORES)])
    e_f_lang = np.concatenate([res.results[k]["out_l"] for k in range(N_CORES)])
    return (e_f, e_f_lang)



# revision 5
# speedup vs baseline: 181.7673x; 181.7673x over previous
"""Trainium2 Bass kernel for nn_H_H_EdgeApplyModule (GNN edge-apply).

Reference computation:
    feat      = concat([n_f[src], s_f, n_f[dst]], 1)          # [E, 3072]
    feat_lang = concat([word2vec[src], word2vec[dst]], 1)     # [E, 600]
    e_f       = relu(feat @ W1 + b1)                          # [E, 256]
    e_f_lang  = relu(feat_lang @ Wl + bl)                     # [E, 256]

Algebraic restructure (cuts FLOPs 2.7x and gather bytes 2.4x):
    W1 = [W1a; W1b; W1c] (rows 0:1024, 1024:2048, 2048:3072)
    Wl = [Wla; Wlb]      (rows 0:300, 300:600)
    Per node, a single combined projection table row (1024 cols, f16):
        T[n] = [P | Pl | Q | Ql]
        P  = n_f@W1a + b1   Pl = w2v@Wla + bl   (src half, bias folded in)
        Q  = n_f@W1c        Ql = w2v@Wlb        (dst half)
    e_f      = relu(P[src] + s_f @ W1b + Q[dst])
    e_f_lang = relu(Pl[src] + Ql[dst])

Distribution (8 cores):
    - Node tables: each core computes a 1/8 node shard of T, then one
      AllGather -> full T in local DRAM.
    - Edges: sharded contiguously; each core handles E/8 edges with
      dma_gather (half-row gather by edge index) + PE matmuls.

Performance structure (vs the first working version):
    - All feature tensors are pre-transposed and cast to f16 on the host
      (layout prep in make_in_maps): no on-chip PE transposes, no f32->f16
      casts, and half the s_f HBM traffic.
    - The s_f @ W1b partial products for all edges are computed into an
      SBUF-resident f16 buffer; this work overlaps the AllGather.
    - Gathered src/dst rows are combined with DVE adds + ACT relu (PE free).
    - Outputs are stored f16 and upcast on the host.
"""

import sys

sys.path.insert(0, "/opt/trn_rl_repo")

import numpy as np

from concourse import bass, bacc, tile, mybir
from concourse.bass_utils import run_bass_kernel_spmd

F32 = mybir.dt.float32
F16 = mybir.dt.float16
I16 = mybir.dt.int16

# ---------------------------------------------------------------- config
N_CORES = 8
N_NODES = 16384
E_TOTAL = 131072
D = 1024          # node/spatial feature dim
DW = 384          # word2vec dim padded 300 -> 384 (3 full 128-chunks)
DX = D + DW       # stacked feature rows (1408)
DOUT = 256
TBL = 1024        # combined table row: [P | Pl | Q | Ql]

E_CORE = E_TOTAL // N_CORES          # 16384
NODE_SHARD = N_NODES // N_CORES      # 2048
BATCH = 1024                         # edges per gather batch
N_BATCH = E_CORE // BATCH            # 16
TPB = BATCH // 128                   # 8 edge tiles per batch
KC_D = D // 128                      # 8 K-chunks for 1024-dim features
KC_W = DW // 128                     # 3 K-chunks for word2vec
IDX_COLS = E_CORE // 16              # int16 index columns per core


def build_kernel(reps=1):
    nc = bacc.Bacc("TRN2", target_bir_lowering=False, debug=False,
                   num_devices=N_CORES)

    # ---------------- I/O ----------------
    xT = nc.declare_dram_parameter("xT", [DX, NODE_SHARD], F16, isOutput=False)
    sfT = nc.declare_dram_parameter("sfT", [D, E_CORE], F16, isOutput=False)
    w_nf = nc.declare_dram_parameter("w_nf", [D, 512], F16, isOutput=False)   # [W1a|W1c]
    w_l = nc.declare_dram_parameter("w_l", [DW, 512], F16, isOutput=False)    # [Wla|Wlb]
    w1b = nc.declare_dram_parameter("w1b", [D, DOUT], F16, isOutput=False)
    bias = nc.declare_dram_parameter("bias_src", [1, 512], F32, isOutput=False)  # [b1|bl]
    ones = nc.declare_dram_parameter("ones", [1, 128], F32, isOutput=False)
    idx_src = nc.declare_dram_parameter("idx_src", [128, IDX_COLS], I16, isOutput=False)
    idx_dst = nc.declare_dram_parameter("idx_dst", [128, IDX_COLS], I16, isOutput=False)
    out_e = nc.declare_dram_parameter("out_e", [E_CORE, DOUT], F16, isOutput=True)
    out_l = nc.declare_dram_parameter("out_l", [E_CORE, DOUT], F16, isOutput=True)

    # ---------------- internal DRAM ----------------
    tsh = nc.dram_tensor("t_shard", [NODE_SHARD, TBL], F16)
    tfull = nc.dram_tensor("t_full", [N_NODES, TBL], F16, addr_space="Shared")

    relu = mybir.ActivationFunctionType.Relu

    with tile.TileContext(nc) as tc:
        with tc.tile_pool(name="const", bufs=1) as cpool:
            # persistent constants in SBUF
            w_nf_sb = cpool.tile([128, KC_D, 512], F16)
            nc.sync.dma_start(w_nf_sb[:], w_nf[:].rearrange("(c p) n -> p c n", p=128))
            w_l_sb = cpool.tile([128, KC_W, 512], F16)
            nc.sync.dma_start(w_l_sb[:], w_l[:].rearrange("(c p) n -> p c n", p=128))
            w1b_sb = cpool.tile([128, KC_D, DOUT], F16)
            nc.sync.dma_start(w1b_sb[:], w1b[:].rearrange("(c p) n -> p c n", p=128))
            ones_sb = cpool.tile([1, 128], F32)
            nc.sync.dma_start(ones_sb[:], ones[:])
            bias_sb = cpool.tile([1, 512], F32)
            nc.sync.dma_start(bias_sb[:], bias[:])
            idx_src_sb = cpool.tile([128, IDX_COLS], I16)
            nc.sync.dma_start(idx_src_sb[:], idx_src[:])
            idx_dst_sb = cpool.tile([128, IDX_COLS], I16)
            nc.sync.dma_start(idx_dst_sb[:], idx_dst[:])

            # broadcast bias to all 128 partitions: psum = ones.T @ bias
            bias_full = cpool.tile([128, 512], F32)
            with tc.tile_pool(name="psum_b", bufs=1, space="PSUM") as pbias:
                pb = pbias.tile([128, 512], F32)
                nc.tensor.matmul(pb[:], ones_sb[:], bias_sb[:], start=True, stop=True)
                nc.vector.tensor_copy(bias_full[:], pb[:])

            for rep in range(reps):
                with (
                    tc.tile_pool(name="p1_x", bufs=2) as p1x,
                    tc.tile_pool(name="p1_o", bufs=2) as p1o,
                    tc.tile_pool(name="p1_ps", bufs=2, space="PSUM") as p1ps,
                ):
                    # ===== phase 1: node table shard =====
                    for g in range(NODE_SHARD // 512):
                        xt = p1x.tile([128, DX // 128, 512], F16, tag="xt")
                        nc.sync.dma_start(
                            xt[:],
                            xT[:, g * 512:(g + 1) * 512].rearrange(
                                "(c p) m -> p c m", p=128))
                        for nt in range(4):
                            sl = slice(nt * 128, (nt + 1) * 128)
                            psA = p1ps.tile([128, 512], F32, tag="psA")
                            psB = p1ps.tile([128, 512], F32, tag="psB")
                            for kc in range(KC_D):
                                nc.tensor.matmul(
                                    psA[:, 0:256], xt[:, kc, sl],
                                    w_nf_sb[:, kc, 0:256],
                                    start=(kc == 0), stop=(kc == KC_D - 1))
                            for kc in range(KC_W):
                                nc.tensor.matmul(
                                    psA[:, 256:512], xt[:, KC_D + kc, sl],
                                    w_l_sb[:, kc, 0:256],
                                    start=(kc == 0), stop=(kc == KC_W - 1))
                            for kc in range(KC_D):
                                nc.tensor.matmul(
                                    psB[:, 0:256], xt[:, kc, sl],
                                    w_nf_sb[:, kc, 256:512],
                                    start=(kc == 0), stop=(kc == KC_D - 1))
                            for kc in range(KC_W):
                                nc.tensor.matmul(
                                    psB[:, 256:512], xt[:, KC_D + kc, sl],
                                    w_l_sb[:, kc, 256:512],
                                    start=(kc == 0), stop=(kc == KC_W - 1))
                            to = p1o.tile([128, TBL], F16, tag="to")
                            nc.vector.tensor_add(to[:, 0:512], psA[:], bias_full[:])
                            nc.scalar.copy(to[:, 512:1024], psB[:])
                            r0 = (g * 4 + nt) * 128
                            nc.sync.dma_start(tsh[r0:r0 + 128, :], to[:])

                # ===== AllGather the combined table =====
                nc.gpsimd.collective_compute(
                    "AllGather", mybir.AluOpType.bypass,
                    replica_groups=[list(range(N_CORES))],
                    ins=[tsh[:]], outs=[tfull[:]])

                with (
                    tc.tile_pool(name="p2_sf", bufs=2) as p2sf,
                    tc.tile_pool(name="p2_pp", bufs=3, space="PSUM") as p2pp,
                    tc.tile_pool(name="p2_part", bufs=1) as p2part,
                    tc.tile_pool(name="p2_g", bufs=2) as p2g,
                    tc.tile_pool(name="p2_o", bufs=2) as p2o,
                ):
                    # ===== phase 2a: s_f @ W1b partials (overlaps AllGather)
                    partial = p2part.tile([128, E_CORE // 128, DOUT], F16)
                    for b in range(N_BATCH):
                        sft = p2sf.tile([128, KC_D, BATCH], F16, tag="sft")
                        nc.sync.dma_start(
                            sft[:],
                            sfT[:, b * BATCH:(b + 1) * BATCH].rearrange(
                                "(c p) e -> p c e", p=128))
                        for h in range(TPB // 2):
                            pp = p2pp.tile([128, 2, DOUT], F32, tag="pp")
                            for u in range(2):
                                t = h * 2 + u
                                for kc in range(KC_D):
                                    nc.tensor.matmul(
                                        pp[:, u, :],
                                        sft[:, kc, t * 128:(t + 1) * 128],
                                        w1b_sb[:, kc, :],
                                        start=(kc == 0), stop=(kc == KC_D - 1))
                            nc.vector.tensor_copy(
                                partial[:, b * TPB + h * 2: b * TPB + h * 2 + 2, :],
                                pp[:])

                    # ===== phase 2b: gather + combine =====
                    for b in range(N_BATCH):
                        c0 = b * (BATCH // 16)
                        cw = BATCH // 16
                        gs = p2g.tile([128, TPB, 512], F16, tag="gs")
                        nc.gpsimd.dma_gather(
                            gs[:], tfull[:, 0:512],
                            idx_src_sb[:, c0:c0 + cw],
                            BATCH, BATCH, 512, elem_step=TBL)
                        gd = p2g.tile([128, TPB, 512], F16, tag="gd")
                        nc.gpsimd.dma_gather(
                            gd[:], tfull[:, 512:1024],
                            idx_dst_sb[:, c0:c0 + cw],
                            BATCH, BATCH, 512, elem_step=TBL)

                        ts_ = p2o.tile([128, TPB, DOUT], F16, tag="tmp")
                        nc.vector.tensor_add(ts_[:], gs[:, :, 0:256], gd[:, :, 0:256])
                        t2 = p2o.tile([128, TPB, DOUT], F16, tag="tmp")
                        nc.vector.tensor_add(
                            t2[:], ts_[:], partial[:, b * TPB:(b + 1) * TPB, :])
                        oe = p2o.tile([128, TPB, DOUT], F16, tag="out")
                        nc.scalar.activation(oe[:], t2[:], relu)
                        tl = p2o.tile([128, TPB, DOUT], F16, tag="tmp")
                        nc.vector.tensor_add(tl[:], gs[:, :, 256:512], gd[:, :, 256:512])
                        ol = p2o.tile([128, TPB, DOUT], F16, tag="out")
                        nc.scalar.activation(ol[:], tl[:], relu)

                        e0 = b * BATCH
                        nc.sync.dma_start(
                            out_e[e0:e0 + BATCH, :].rearrange(
                                "(t p) n -> p t n", p=128), oe[:])
                        nc.sync.dma_start(
                            out_l[e0:e0 + BATCH, :].rearrange(
                                "(t p) n -> p t n", p=128), ol[:])

    nc.compile()
    return nc


# ---------------------------------------------------------------- host side
def _wrap_idx(ix):
    """int16 index layout for dma_gather: idx j of a batch sits at
    (partition j%16, column j//16); 16-row block replicated to 128."""
    e = ix.shape[0]
    n_b = e // BATCH
    cols = BATCH // 16
    arr = np.zeros((16, e // 16), dtype=np.int16)
    for b in range(n_b):
        blk = ix[b * BATCH:(b + 1) * BATCH].astype(np.int16).reshape(cols, 16).T
        arr[:, b * cols:(b + 1) * cols] = blk
    return np.ascontiguousarray(np.tile(arr, (8, 1)))


_NC_CACHE = {}


def make_in_maps(n_f, word2vec, s_f, W1, b1, Wl, bl, src, dst):
    n_f = np.asarray(n_f, dtype=np.float32)
    word2vec = np.asarray(word2vec, dtype=np.float32)
    s_f = np.asarray(s_f, dtype=np.float32)
    W1 = np.asarray(W1, dtype=np.float32)
    Wl = np.asarray(Wl, dtype=np.float32)
    b1 = np.asarray(b1, dtype=np.float32)
    bl = np.asarray(bl, dtype=np.float32)
    src = np.asarray(src)
    dst = np.asarray(dst)

    w_nf_h = np.ascontiguousarray(
        np.concatenate([W1[0:D], W1[2 * D:3 * D]], axis=1)).astype(np.float16)
    w_l_h = np.zeros((DW, 512), np.float16)
    w_l_h[:300, 0:256] = Wl[0:300]
    w_l_h[:300, 256:512] = Wl[300:600]
    w1b_h = np.ascontiguousarray(W1[D:2 * D]).astype(np.float16)
    bias_h = np.concatenate([b1, bl])[None, :].astype(np.float32)
    ones_h = np.ones((1, 128), np.float32)

    xT_full = np.empty((DX, N_NODES), np.float16)
    xT_full[:D] = n_f.T
    xT_full[D:D + 300] = word2vec.T
    xT_full[D + 300:] = 0.0

    in_maps = []
    for k in range(N_CORES):
        es, ee = k * E_CORE, (k + 1) * E_CORE
        ns, ne = k * NODE_SHARD, (k + 1) * NODE_SHARD
        in_maps.append({
            "xT": np.ascontiguousarray(xT_full[:, ns:ne]),
            "sfT": np.ascontiguousarray(s_f[es:ee].T.astype(np.float16)),
            "w_nf": w_nf_h,
            "w_l": w_l_h,
            "w1b": w1b_h,
            "bias_src": bias_h,
            "ones": ones_h,
            "idx_src": _wrap_idx(src[es:ee]),
            "idx_dst": _wrap_idx(dst[es:ee]),
        })

    return in_maps


def kernel(n_f, word2vec, s_f, W1, b1, Wl, bl, src, dst):
    if "nc" not in _NC_CACHE:
        _NC_CACHE["nc"] = build_kernel()
    nc = _NC_CACHE["nc"]
    in_maps = make_in_maps(n_f, word2vec, s_f, W1, b1, Wl, bl, src, dst)
    res = run_bass_kernel_spmd(nc, in_maps, list(range(N_CORES)))
    _NC_CACHE["last_results"] = res
    e_f = np.concatenate(
        [res.results[k]["out_e"] for k in range(N_CORES)]).astype(np.float32)
    e_f_lang = np.concatenate(
        [res.results[k]["out_l"] for k in range(N_CORES)]).astype(np.float32)
    return (e_f, e_f_lang)
